# revision 14
# baseline (speedup 1.0000x reference)
"""Trainium2 Bass kernel for nn_Attention_80805514707533.

Recurrent attention scan: B=512, T=512, C=64, H=128.
Sharding: H across 8 cores (16 heads each); full batch B=512 rides the
matmul moving dimension. C=64 lives on partitions; heads are packed in
pairs (2 x 64 = 128 partitions) with block-diagonal stationary weights.

bf16 datapath (PE 1 cyc/row + fast weight load), fp32 PSUM accumulation,
fast approximate reciprocal for the softmax denominator. Small-K matmuls
(wi: K=64, bc: K=16) are issued in pairs at different PE row groups so
they execute concurrently in the 128x128 array.

Per step t (per core, j = head-pair 0..7):
  pre[j]  = Wi_cat[j].T @ xT_t  +  Wa_blk[j].T @ att[:,j,:]      (PSUM)
  v[j]    = tanh(pre[j])                                          (ACT)
  e[j]    = We_blk[j].T @ v[j]                                    (PSUM)
  u[j]    = exp(e[j])                                             (ACT)
  S       = sum_c u   via ones16 matmuls  -> [16, B]              (PE)
  rS      = 1/S  (approx, fp32)                                   (DVE)
  att'    = u * bcast(rS)   (bcast via selector matmul)           (PE+DVE)
  p       = att' * x                                              (GPSIMD)
  out_t   = sum_c p  via ones16 matmuls -> [16, B] -> DRAM        (PE+DVE)
"""

import numpy as np
import ml_dtypes

B, T, C, H = 512, 512, 64, 128
NCORES = 8
HL = H // NCORES          # heads per core = 16
NPAIR = HL // 2           # head pairs per core = 8


def _build_nc(t_steps: int):
    import concourse.bass as bass
    import concourse.bacc as bacc
    import concourse.mybir as mybir
    import concourse.tile as tile
    from contextlib import ExitStack

    fp32 = mybir.dt.float32
    bf16 = mybir.dt.bfloat16
    nc = bacc.Bacc("TRN2", target_bir_lowering=False, debug=False,
                   num_devices=NCORES)

    xT_d = nc.dram_tensor("xT", [C, t_steps, B], bf16, kind="ExternalInput")
    wi_d = nc.dram_tensor("wi", [C, NPAIR, 128], bf16, kind="ExternalInput")
    wa_d = nc.dram_tensor("wa", [128, NPAIR, 128], bf16, kind="ExternalInput")
    we_d = nc.dram_tensor("we", [128, NPAIR, 128], bf16, kind="ExternalInput")
    on_d = nc.dram_tensor("ones16", [128, NPAIR, 48], bf16, kind="ExternalInput")
    sel_d = nc.dram_tensor("sel", [HL, NPAIR, 128], bf16, kind="ExternalInput")
    out_d = nc.dram_tensor("out", [t_steps, HL, B], fp32, kind="ExternalOutput")

    with ExitStack() as ctx:
        ctx.enter_context(nc.allow_low_precision(reason="bf16 datapath"))
        tc = ctx.enter_context(tile.TileContext(nc))
        singles = ctx.enter_context(tc.tile_pool(name="singles", bufs=1))
        state = ctx.enter_context(tc.tile_pool(name="state", bufs=3))
        xpool = ctx.enter_context(tc.tile_pool(name="xpool", bufs=4))
        vpool = ctx.enter_context(tc.tile_pool(name="vpool", bufs=3))
        upool = ctx.enter_context(tc.tile_pool(name="upool", bufs=3))
        wpool = ctx.enter_context(tc.tile_pool(name="wpool", bufs=3))
        spool = ctx.enter_context(tc.tile_pool(name="spool", bufs=3))
        opool = ctx.enter_context(tc.tile_pool(name="opool", bufs=3))
        ps_pre = ctx.enter_context(tc.tile_pool(name="ps_pre", bufs=2, space="PSUM"))
        ps_e = ctx.enter_context(tc.tile_pool(name="ps_e", bufs=2, space="PSUM"))
        ps_sn = ctx.enter_context(tc.tile_pool(name="ps_sn", bufs=1, space="PSUM"))
        ps_bc = ctx.enter_context(tc.tile_pool(name="ps_bc", bufs=2, space="PSUM"))

        wi_sb = singles.tile([C, NPAIR, 128], bf16)
        wa_sb = singles.tile([128, NPAIR, 128], bf16)
        we_sb = singles.tile([128, NPAIR, 128], bf16)
        on_sb = singles.tile([128, NPAIR, 48], bf16)
        sel_sb = singles.tile([HL, NPAIR, 128], bf16)
        nc.sync.dma_start(out=wi_sb, in_=wi_d[:])
        nc.sync.dma_start(out=wa_sb, in_=wa_d[:])
        nc.sync.dma_start(out=we_sb, in_=we_d[:])
        nc.sync.dma_start(out=on_sb, in_=on_d[:])
        nc.sync.dma_start(out=sel_sb, in_=sel_d[:])

        att = state.tile([128, NPAIR, B], bf16, tag="att")
        nc.vector.memset(att, 1.0 / C)

        for t in range(t_steps):
            xdup = xpool.tile([128, B], bf16)
            nc.sync.dma_start(out=xdup[0:C, :], in_=xT_d[:, t, :])
            nc.sync.dma_start(out=xdup[C:128, :], in_=xT_d[:, t, :])

            v_sb = vpool.tile([128, NPAIR, B], bf16)
            for j in range(NPAIR):
                pre = ps_pre.tile([128, B], fp32)
                nc.tensor.matmul(pre, wi_sb[:, j, :], xdup[0:C, :],
                                 start=True, stop=False)
                nc.tensor.matmul(pre, wa_sb[:, j, :], att[:, j, :],
                                 start=False, stop=True)
                nc.scalar.activation(v_sb[:, j, :], pre,
                                     mybir.ActivationFunctionType.Tanh)

            u_sb = upool.tile([128, NPAIR, B], bf16)
            for j in range(NPAIR):
                e = ps_e.tile([128, B], fp32)
                nc.tensor.matmul(e, we_sb[:, j, :], v_sb[:, j, :],
                                 start=True, stop=True)
                nc.scalar.activation(u_sb[:, j, :], e,
                                     mybir.ActivationFunctionType.Exp)

            # w = u * x  (numerator operand; off the recurrence, on GPSIMD)
            w_sb = wpool.tile([128, NPAIR, B], bf16)
            for j in range(NPAIR):
                nc.gpsimd.tensor_mul(w_sb[:, j, :], u_sb[:, j, :], xdup)

            S_ps = ps_sn.tile([HL, B], fp32, tag="S")
            for j in range(NPAIR):
                nc.tensor.matmul(S_ps, on_sb[:, j, 0:HL], u_sb[:, j, :],
                                 start=(j == 0), stop=(j == NPAIR - 1))

            rS32 = spool.tile([HL, B], fp32, tag="rS32")
            nc.vector.reciprocal_approx_fast(out=rS32, in_=S_ps)
            rSb = spool.tile([HL, B], bf16, tag="rSb")
            nc.vector.tensor_copy(rSb, rS32)

            num_ps = ps_sn.tile([HL, B], fp32, tag="num")
            for j in range(NPAIR):
                nc.tensor.matmul(num_ps, on_sb[:, j, 32:48], w_sb[:, j, :],
                                 start=(j == 0), stop=(j == NPAIR - 1))

            outb = opool.tile([HL, B], fp32)
            nc.vector.tensor_mul(outb, num_ps, rS32)
            nc.sync.dma_start(out=out_d[t], in_=outb)

            # att' = u * bcast(rS); bcast via selector matmul per pair
            att_new = state.tile([128, NPAIR, B], bf16, tag="att")
            for j in range(NPAIR):
                bc = ps_bc.tile([128, B], fp32)
                nc.tensor.matmul(bc, sel_sb[:, j, :], rSb,
                                 start=True, stop=True)
                nc.vector.tensor_mul(att_new[:, j, :], u_sb[:, j, :], bc)
            att = att_new

    nc.compile()
    return nc


def _host_prep(x, weight_att, weight_input, weight_e):
    """Build per-core input maps (host-side layout prep)."""
    bf = ml_dtypes.bfloat16
    xT = np.ascontiguousarray(x.transpose(2, 1, 0)).astype(bf)  # [C, T, B]

    in_maps = []
    for g in range(NCORES):
        h0 = g * HL
        wi = np.zeros((C, NPAIR, 128), np.float32)
        wa = np.zeros((128, NPAIR, 128), np.float32)
        we = np.zeros((128, NPAIR, 128), np.float32)
        on = np.zeros((128, NPAIR, 48), np.float32)
        sel = np.zeros((HL, NPAIR, 128), np.float32)
        for j in range(NPAIR):
            ha, hb = h0 + 2 * j, h0 + 2 * j + 1
            # lhsT[k, m] = W[h, m, k]
            wi[:, j, 0:C] = weight_input[ha].T
            wi[:, j, C:128] = weight_input[hb].T
            wa[0:C, j, 0:C] = weight_att[ha].T
            wa[C:128, j, C:128] = weight_att[hb].T
            we[0:C, j, 0:C] = weight_e[ha].T
            we[C:128, j, C:128] = weight_e[hb].T
            on[0:C, j, 2 * j] = 1.0
            on[C:128, j, 2 * j + 1] = 1.0
            on[0:C, j, 32 + 2 * j] = 1.0
            on[C:128, j, 32 + 2 * j + 1] = 1.0
            sel[2 * j, j, 0:C] = 1.0
            sel[2 * j + 1, j, C:128] = 1.0
        in_maps.append({
            "xT": xT, "wi": wi.astype(bf), "wa": wa.astype(bf),
            "we": we.astype(bf), "ones16": on.astype(bf),
            "sel": sel.astype(bf),
        })
    return in_maps


def run(x, weight_att, weight_input, weight_e, t_steps=T, trace=False):
    from concourse.bass_utils import run_bass_kernel_spmd

    nc = _build_nc(t_steps)
    in_maps = _host_prep(x, weight_att, weight_input, weight_e)
    if t_steps != T:
        for m in in_maps:
            m["xT"] = np.ascontiguousarray(m["xT"][:, :t_steps, :])
    res = run_bass_kernel_spmd(nc, in_maps, list(range(NCORES)), trace=trace)

    # results[g]["out"]: [t_steps, HL, B] -> out[b, t, g*HL + i]
    out = np.empty((B, t_steps, H), np.float32)
    for g in range(NCORES):
        og = res.results[g]["out"]
        out[:, :, g * HL:(g + 1) * HL] = og.transpose(2, 0, 1)
    return out, res


def kernel(x, weight_att, weight_input, weight_e):
    out, _ = run(x, weight_att, weight_input, weight_e)
    return out


# revision 16
# speedup vs baseline: 1.0006x; 1.0006x over previous
"""Trainium2 Bass kernel for nn_Attention_80805514707533.

Recurrent attention scan: B=512, T=512, C=64, H=128.
Sharding: H across 8 cores (16 heads each); full batch B=512 rides the
matmul moving dimension. C=64 lives on partitions; heads are packed in
pairs (2 x 64 = 128 partitions) with block-diagonal stationary weights.

bf16 datapath (PE 1 cyc/row + fast weight load), fp32 PSUM accumulation,
fast approximate reciprocal for the softmax denominator. Small-K matmuls
(wi: K=64, bc: K=16) are issued in pairs at different PE row groups so
they execute concurrently in the 128x128 array.

Per step t (per core, j = head-pair 0..7):
  pre[j]  = Wi_cat[j].T @ xT_t  +  Wa_blk[j].T @ att[:,j,:]      (PSUM)
  v[j]    = tanh(pre[j])                                          (ACT)
  e[j]    = We_blk[j].T @ v[j]                                    (PSUM)
  u[j]    = exp(e[j])                                             (ACT)
  S       = sum_c u   via ones16 matmuls  -> [16, B]              (PE)
  rS      = 1/S  (approx, fp32)                                   (DVE)
  att'    = u * bcast(rS)   (bcast via selector matmul)           (PE+DVE)
  p       = att' * x                                              (GPSIMD)
  out_t   = sum_c p  via ones16 matmuls -> [16, B] -> DRAM        (PE+DVE)
"""

import numpy as np
import ml_dtypes

B, T, C, H = 512, 512, 64, 128
NCORES = 8
HL = H // NCORES          # heads per core = 16
NPAIR = HL // 2           # head pairs per core = 8


def _build_nc(t_steps: int):
    import concourse.bass as bass
    import concourse.bacc as bacc
    import concourse.mybir as mybir
    import concourse.tile as tile
    from contextlib import ExitStack

    fp32 = mybir.dt.float32
    bf16 = mybir.dt.bfloat16
    nc = bacc.Bacc("TRN2", target_bir_lowering=False, debug=False,
                   num_devices=NCORES)

    xT_d = nc.dram_tensor("xT", [C, t_steps, B], bf16, kind="ExternalInput")
    wi_d = nc.dram_tensor("wi", [C, NPAIR, 128], bf16, kind="ExternalInput")
    wa_d = nc.dram_tensor("wa", [128, NPAIR, 128], bf16, kind="ExternalInput")
    we_d = nc.dram_tensor("we", [128, NPAIR, 128], bf16, kind="ExternalInput")
    on_d = nc.dram_tensor("ones16", [128, NPAIR, 48], bf16, kind="ExternalInput")
    sel_d = nc.dram_tensor("sel", [HL, NPAIR, 128], bf16, kind="ExternalInput")
    out_d = nc.dram_tensor("out", [t_steps, HL, B], fp32, kind="ExternalOutput")

    with ExitStack() as ctx:
        ctx.enter_context(nc.allow_low_precision(reason="bf16 datapath"))
        tc = ctx.enter_context(tile.TileContext(nc))
        singles = ctx.enter_context(tc.tile_pool(name="singles", bufs=1))
        state = ctx.enter_context(tc.tile_pool(name="state", bufs=3))
        xpool = ctx.enter_context(tc.tile_pool(name="xpool", bufs=4))
        vpool = ctx.enter_context(tc.tile_pool(name="vpool", bufs=3))
        upool = ctx.enter_context(tc.tile_pool(name="upool", bufs=3))
        wpool = ctx.enter_context(tc.tile_pool(name="wpool", bufs=3))
        spool = ctx.enter_context(tc.tile_pool(name="spool", bufs=3))
        opool = ctx.enter_context(tc.tile_pool(name="opool", bufs=3))
        ps_pre = ctx.enter_context(tc.tile_pool(name="ps_pre", bufs=2, space="PSUM"))
        ps_e = ctx.enter_context(tc.tile_pool(name="ps_e", bufs=2, space="PSUM"))
        ps_sn = ctx.enter_context(tc.tile_pool(name="ps_sn", bufs=1, space="PSUM"))
        ps_bc = ctx.enter_context(tc.tile_pool(name="ps_bc", bufs=2, space="PSUM"))

        wi_sb = singles.tile([C, NPAIR, 128], bf16)
        wa_sb = singles.tile([128, NPAIR, 128], bf16)
        we_sb = singles.tile([128, NPAIR, 128], bf16)
        on_sb = singles.tile([128, NPAIR, 48], bf16)
        sel_sb = singles.tile([HL, NPAIR, 128], bf16)
        nc.sync.dma_start(out=wi_sb, in_=wi_d[:])
        nc.sync.dma_start(out=wa_sb, in_=wa_d[:])
        nc.sync.dma_start(out=we_sb, in_=we_d[:])
        nc.sync.dma_start(out=on_sb, in_=on_d[:])
        nc.sync.dma_start(out=sel_sb, in_=sel_d[:])

        att = state.tile([128, NPAIR, B], bf16, tag="att")
        nc.vector.memset(att, 1.0 / C)

        for t in range(t_steps):
            xdup = xpool.tile([128, B], bf16)
            nc.sync.dma_start(out=xdup[0:C, :], in_=xT_d[:, t, :])
            nc.sync.dma_start(out=xdup[C:128, :], in_=xT_d[:, t, :])

            v_sb = vpool.tile([128, NPAIR, B], bf16)
            for j in range(NPAIR):
                pre = ps_pre.tile([128, B], fp32)
                nc.tensor.matmul(pre, wi_sb[:, j, :], xdup[0:C, :],
                                 start=True, stop=False)
                nc.tensor.matmul(pre, wa_sb[:, j, :], att[:, j, :],
                                 start=False, stop=True)
                nc.scalar.activation(v_sb[:, j, :], pre,
                                     mybir.ActivationFunctionType.Tanh)

            u_sb = upool.tile([128, NPAIR, B], bf16)
            for j in range(NPAIR):
                e = ps_e.tile([128, B], fp32)
                nc.tensor.matmul(e, we_sb[:, j, :], v_sb[:, j, :],
                                 start=True, stop=True)
                nc.scalar.activation(u_sb[:, j, :], e,
                                     mybir.ActivationFunctionType.Exp)

            # w = u * x  (numerator operand; off the recurrence, on GPSIMD)
            w_sb = wpool.tile([128, NPAIR, B], bf16)
            for j in range(NPAIR):
                nc.gpsimd.tensor_mul(w_sb[:, j, :], u_sb[:, j, :], xdup)

            S_ps = ps_sn.tile([HL, B], fp32, tag="S")
            for j in range(NPAIR):
                nc.tensor.matmul(S_ps, on_sb[:, j, 0:HL], u_sb[:, j, :],
                                 start=(j == 0), stop=(j == NPAIR - 1))

            rS32 = spool.tile([HL, B], fp32, tag="rS32")
            nc.vector.reciprocal_approx_fast(out=rS32, in_=S_ps)
            rSb = spool.tile([HL, B], bf16, tag="rSb")
            nc.vector.tensor_copy(rSb, rS32)

            num_ps = ps_sn.tile([HL, B], fp32, tag="num")
            for j in range(NPAIR):
                nc.tensor.matmul(num_ps, on_sb[:, j, 32:48], w_sb[:, j, :],
                                 start=(j == 0), stop=(j == NPAIR - 1))

            # att' = u * bcast(rS); bcast via selector matmul per pair
            att_new = state.tile([128, NPAIR, B], bf16, tag="att")
            for j in range(NPAIR):
                bc = ps_bc.tile([128, B], fp32)
                nc.tensor.matmul(bc, sel_sb[:, j, :], rSb,
                                 start=True, stop=True)
                nc.vector.tensor_mul(att_new[:, j, :], u_sb[:, j, :], bc)
            att = att_new

            # out-mul emitted after the att'-muls: it depends on the late
            # num matmuls and must not block the critical DVE chain.
            outb = opool.tile([HL, B], fp32)
            nc.vector.tensor_mul(outb, num_ps, rS32)
            nc.sync.dma_start(out=out_d[t], in_=outb)

    nc.compile()
    return nc


def _host_prep(x, weight_att, weight_input, weight_e):
    """Build per-core input maps (host-side layout prep)."""
    bf = ml_dtypes.bfloat16
    xT = np.ascontiguousarray(x.transpose(2, 1, 0)).astype(bf)  # [C, T, B]

    in_maps = []
    for g in range(NCORES):
        h0 = g * HL
        wi = np.zeros((C, NPAIR, 128), np.float32)
        wa = np.zeros((128, NPAIR, 128), np.float32)
        we = np.zeros((128, NPAIR, 128), np.float32)
        on = np.zeros((128, NPAIR, 48), np.float32)
        sel = np.zeros((HL, NPAIR, 128), np.float32)
        for j in range(NPAIR):
            ha, hb = h0 + 2 * j, h0 + 2 * j + 1
            # lhsT[k, m] = W[h, m, k]
            wi[:, j, 0:C] = weight_input[ha].T
            wi[:, j, C:128] = weight_input[hb].T
            wa[0:C, j, 0:C] = weight_att[ha].T
            wa[C:128, j, C:128] = weight_att[hb].T
            we[0:C, j, 0:C] = weight_e[ha].T
            we[C:128, j, C:128] = weight_e[hb].T
            on[0:C, j, 2 * j] = 1.0
            on[C:128, j, 2 * j + 1] = 1.0
            on[0:C, j, 32 + 2 * j] = 1.0
            on[C:128, j, 32 + 2 * j + 1] = 1.0
            sel[2 * j, j, 0:C] = 1.0
            sel[2 * j + 1, j, C:128] = 1.0
        in_maps.append({
            "xT": xT, "wi": wi.astype(bf), "wa": wa.astype(bf),
            "we": we.astype(bf), "ones16": on.astype(bf),
            "sel": sel.astype(bf),
        })
    return in_maps


def run(x, weight_att, weight_input, weight_e, t_steps=T, trace=False):
    from concourse.bass_utils import run_bass_kernel_spmd

    nc = _build_nc(t_steps)
    in_maps = _host_prep(x, weight_att, weight_input, weight_e)
    if t_steps != T:
        for m in in_maps:
            m["xT"] = np.ascontiguousarray(m["xT"][:, :t_steps, :])
    res = run_bass_kernel_spmd(nc, in_maps, list(range(NCORES)), trace=trace)

    # results[g]["out"]: [t_steps, HL, B] -> out[b, t, g*HL + i]
    out = np.empty((B, t_steps, H), np.float32)
    for g in range(NCORES):
        og = res.results[g]["out"]
        out[:, :, g * HL:(g + 1) * HL] = og.transpose(2, 0, 1)
    return out, res


def kernel(x, weight_att, weight_input, weight_e):
    out, _ = run(x, weight_att, weight_input, weight_e)
    return out


# revision 20
# speedup vs baseline: 1.0153x; 1.0147x over previous
"""Trainium2 Bass kernel for nn_Attention_80805514707533.

Recurrent attention scan: B=512, T=512, C=64, H=128.
Sharding: H across 8 cores (16 heads each); full batch B=512 rides the
matmul moving dimension. C=64 lives on partitions; heads are packed in
pairs (2 x 64 = 128 partitions) with block-diagonal stationary weights.

bf16 datapath (PE 1 cyc/row + fast weight load), fp32 PSUM accumulation,
fast approximate reciprocal for the softmax denominator. Small-K matmuls
(wi: K=64, bc: K=16) are issued in pairs at different PE row groups so
they execute concurrently in the 128x128 array.

Per step t (per core, j = head-pair 0..7):
  pre[j]  = Wi_cat[j].T @ xT_t  +  Wa_blk[j].T @ att[:,j,:]      (PSUM)
  v[j]    = tanh(pre[j])                                          (ACT)
  e[j]    = We_blk[j].T @ v[j]                                    (PSUM)
  u[j]    = exp(e[j])                                             (ACT)
  S       = sum_c u   via ones16 matmuls  -> [16, B]              (PE)
  rS      = 1/S  (approx, fp32)                                   (DVE)
  att'    = u * bcast(rS)   (bcast via selector matmul)           (PE+DVE)
  p       = att' * x                                              (GPSIMD)
  out_t   = sum_c p  via ones16 matmuls -> [16, B] -> DRAM        (PE+DVE)
"""

import numpy as np
import ml_dtypes

B, T, C, H = 512, 512, 64, 128
NCORES = 8
HL = H // NCORES          # heads per core = 16
NPAIR = HL // 2           # head pairs per core = 8


def _build_nc(t_steps: int):
    import concourse.bass as bass
    import concourse.bacc as bacc
    import concourse.mybir as mybir
    import concourse.tile as tile
    from contextlib import ExitStack

    fp32 = mybir.dt.float32
    bf16 = mybir.dt.bfloat16
    nc = bacc.Bacc("TRN2", target_bir_lowering=False, debug=False,
                   num_devices=NCORES)

    xT_d = nc.dram_tensor("xT", [C, t_steps, B], bf16, kind="ExternalInput")
    wi_d = nc.dram_tensor("wi", [C, NPAIR, 128], bf16, kind="ExternalInput")
    wa_d = nc.dram_tensor("wa", [128, NPAIR, 128], bf16, kind="ExternalInput")
    we_d = nc.dram_tensor("we", [128, NPAIR, 128], bf16, kind="ExternalInput")
    on_d = nc.dram_tensor("ones16", [128, NPAIR, 48], bf16, kind="ExternalInput")
    sel_d = nc.dram_tensor("sel", [HL, NPAIR, 128], bf16, kind="ExternalInput")
    out_d = nc.dram_tensor("out", [t_steps, HL, B], fp32, kind="ExternalOutput")

    with ExitStack() as ctx:
        ctx.enter_context(nc.allow_low_precision(reason="bf16 datapath"))
        tc = ctx.enter_context(tile.TileContext(nc))
        singles = ctx.enter_context(tc.tile_pool(name="singles", bufs=1))
        state = ctx.enter_context(tc.tile_pool(name="state", bufs=3))
        xpool = ctx.enter_context(tc.tile_pool(name="xpool", bufs=4))
        vpool = ctx.enter_context(tc.tile_pool(name="vpool", bufs=3))
        upool = ctx.enter_context(tc.tile_pool(name="upool", bufs=3))
        wpool = ctx.enter_context(tc.tile_pool(name="wpool", bufs=3))
        spool = ctx.enter_context(tc.tile_pool(name="spool", bufs=3))
        opool = ctx.enter_context(tc.tile_pool(name="opool", bufs=3))
        ps_pre = ctx.enter_context(tc.tile_pool(name="ps_pre", bufs=2, space="PSUM"))
        ps_e = ctx.enter_context(tc.tile_pool(name="ps_e", bufs=2, space="PSUM"))
        ps_sn = ctx.enter_context(tc.tile_pool(name="ps_sn", bufs=1, space="PSUM"))
        ps_bc = ctx.enter_context(tc.tile_pool(name="ps_bc", bufs=2, space="PSUM"))

        wi_sb = singles.tile([C, NPAIR, 128], bf16)
        wa_sb = singles.tile([128, NPAIR, 128], bf16)
        we_sb = singles.tile([128, NPAIR, 128], bf16)
        on_sb = singles.tile([128, NPAIR, 48], bf16)
        sel_sb = singles.tile([HL, NPAIR, 128], bf16)
        nc.sync.dma_start(out=wi_sb, in_=wi_d[:])
        nc.sync.dma_start(out=wa_sb, in_=wa_d[:])
        nc.sync.dma_start(out=we_sb, in_=we_d[:])
        nc.sync.dma_start(out=on_sb, in_=on_d[:])
        nc.sync.dma_start(out=sel_sb, in_=sel_d[:])

        att = state.tile([128, NPAIR, B], bf16, tag="att")
        nc.vector.memset(att, 1.0 / C)

        for t in range(t_steps):
            xdup = xpool.tile([128, B], bf16)
            nc.sync.dma_start(out=xdup[0:C, :], in_=xT_d[:, t, :])
            nc.sync.dma_start(out=xdup[C:128, :], in_=xT_d[:, t, :])

            v_sb = vpool.tile([128, NPAIR, B], bf16)
            for j in range(NPAIR):
                pre = ps_pre.tile([128, B], fp32)
                nc.tensor.matmul(pre, wi_sb[:, j, :], xdup[0:C, :],
                                 start=True, stop=False)
                nc.tensor.matmul(pre, wa_sb[:, j, :], att[:, j, :],
                                 start=False, stop=True)
                nc.scalar.activation(v_sb[:, j, :], pre,
                                     mybir.ActivationFunctionType.Tanh)

            u_sb = upool.tile([128, NPAIR, B], bf16)
            for j in range(NPAIR):
                e = ps_e.tile([128, B], fp32)
                nc.tensor.matmul(e, we_sb[:, j, :], v_sb[:, j, :],
                                 start=True, stop=True)
                nc.scalar.activation(u_sb[:, j, :], e,
                                     mybir.ActivationFunctionType.Exp)

            # w = u * x  (numerator operand; off the recurrence, on GPSIMD)
            w_sb = wpool.tile([128, NPAIR, B], bf16)
            for j in range(NPAIR):
                nc.gpsimd.tensor_mul(w_sb[:, j, :], u_sb[:, j, :], xdup)

            S_ps = ps_sn.tile([HL, B], fp32, tag="S")
            for j in range(NPAIR):
                nc.tensor.matmul(S_ps, on_sb[:, j, 0:HL], u_sb[:, j, :],
                                 start=(j == 0), stop=(j == NPAIR - 1))

            # approx 1/S emitted directly as bf16 (the fp32 bit-trick is on
            # the input side only; the write port downcasts) - skips a cast.
            from concourse.dve_ops import (RECIP_APPROX_FAST_CONSTS,
                                           RECIPROCAL_APPROX_FAST)
            _rc = RECIP_APPROX_FAST_CONSTS
            rSb = spool.tile([HL, B], bf16, tag="rSb")
            nc.vector._custom_dve(RECIPROCAL_APPROX_FAST, out=rSb, in0=S_ps,
                                  s0=_rc["s0"], s1=_rc["s1"], imm2=_rc["imm2"])

            num_ps = ps_sn.tile([HL, B], fp32, tag="num")
            for j in range(NPAIR):
                nc.tensor.matmul(num_ps, on_sb[:, j, 32:48], w_sb[:, j, :],
                                 start=(j == 0), stop=(j == NPAIR - 1))

            # att' = u * bcast(rS); bcast via selector matmul per pair
            att_new = state.tile([128, NPAIR, B], bf16, tag="att")
            for j in range(NPAIR):
                bc = ps_bc.tile([128, B], fp32)
                nc.tensor.matmul(bc, sel_sb[:, j, :], rSb,
                                 start=True, stop=True)
                nc.vector.tensor_mul(att_new[:, j, :], u_sb[:, j, :], bc)
            att = att_new

            # out-mul emitted after the att'-muls: it depends on the late
            # num matmuls and must not block the critical DVE chain.
            outb = opool.tile([HL, B], fp32)
            nc.vector.tensor_mul(outb, num_ps, rSb)
            nc.sync.dma_start(out=out_d[t], in_=outb)

    nc.compile()
    return nc


def _host_prep(x, weight_att, weight_input, weight_e):
    """Build per-core input maps (host-side layout prep)."""
    bf = ml_dtypes.bfloat16
    xT = np.ascontiguousarray(x.transpose(2, 1, 0)).astype(bf)  # [C, T, B]

    in_maps = []
    for g in range(NCORES):
        h0 = g * HL
        wi = np.zeros((C, NPAIR, 128), np.float32)
        wa = np.zeros((128, NPAIR, 128), np.float32)
        we = np.zeros((128, NPAIR, 128), np.float32)
        on = np.zeros((128, NPAIR, 48), np.float32)
        sel = np.zeros((HL, NPAIR, 128), np.float32)
        for j in range(NPAIR):
            ha, hb = h0 + 2 * j, h0 + 2 * j + 1
            # lhsT[k, m] = W[h, m, k]
            wi[:, j, 0:C] = weight_input[ha].T
            wi[:, j, C:128] = weight_input[hb].T
            wa[0:C, j, 0:C] = weight_att[ha].T
            wa[C:128, j, C:128] = weight_att[hb].T
            we[0:C, j, 0:C] = weight_e[ha].T
            we[C:128, j, C:128] = weight_e[hb].T
            on[0:C, j, 2 * j] = 1.0
            on[C:128, j, 2 * j + 1] = 1.0
            on[0:C, j, 32 + 2 * j] = 1.0
            on[C:128, j, 32 + 2 * j + 1] = 1.0
            sel[2 * j, j, 0:C] = 1.0
            sel[2 * j + 1, j, C:128] = 1.0
        in_maps.append({
            "xT": xT, "wi": wi.astype(bf), "wa": wa.astype(bf),
            "we": we.astype(bf), "ones16": on.astype(bf),
            "sel": sel.astype(bf),
        })
    return in_maps


def run(x, weight_att, weight_input, weight_e, t_steps=T, trace=False):
    from concourse.bass_utils import run_bass_kernel_spmd

    nc = _build_nc(t_steps)
    in_maps = _host_prep(x, weight_att, weight_input, weight_e)
    if t_steps != T:
        for m in in_maps:
            m["xT"] = np.ascontiguousarray(m["xT"][:, :t_steps, :])
    res = run_bass_kernel_spmd(nc, in_maps, list(range(NCORES)), trace=trace)

    # results[g]["out"]: [t_steps, HL, B] -> out[b, t, g*HL + i]
    out = np.empty((B, t_steps, H), np.float32)
    for g in range(NCORES):
        og = res.results[g]["out"]
        out[:, :, g * HL:(g + 1) * HL] = og.transpose(2, 0, 1)
    return out, res


def kernel(x, weight_att, weight_input, weight_e):
    out, _ = run(x, weight_att, weight_input, weight_e)
    return out


# revision 21
# speedup vs baseline: 1.0700x; 1.0539x over previous
"""Trainium2 Bass kernel for nn_Attention_80805514707533.

Recurrent attention scan: B=512, T=512, C=64, H=128.
Sharding: H across 8 cores (16 heads each); full batch B=512 rides the
matmul moving dimension. C=64 lives on partitions; heads are packed in
pairs (2 x 64 = 128 partitions) with block-diagonal stationary weights.

bf16 datapath (PE 1 cyc/row + fast weight load), fp32 PSUM accumulation,
fast approximate reciprocal for the softmax denominator. Small-K matmuls
(wi: K=64, bc: K=16) are issued in pairs at different PE row groups so
they execute concurrently in the 128x128 array.

Per step t (per core, j = head-pair 0..7):
  pre[j]  = Wi_cat[j].T @ xT_t  +  Wa_blk[j].T @ att[:,j,:]      (PSUM)
  v[j]    = tanh(pre[j])                                          (ACT)
  e[j]    = We_blk[j].T @ v[j]                                    (PSUM)
  u[j]    = exp(e[j])                                             (ACT)
  S       = sum_c u   via ones16 matmuls  -> [16, B]              (PE)
  rS      = 1/S  (approx, fp32)                                   (DVE)
  att'    = u * bcast(rS)   (bcast via selector matmul)           (PE+DVE)
  p       = att' * x                                              (GPSIMD)
  out_t   = sum_c p  via ones16 matmuls -> [16, B] -> DRAM        (PE+DVE)
"""

import numpy as np
import ml_dtypes

B, T, C, H = 512, 512, 64, 128
NCORES = 8
HL = H // NCORES          # heads per core = 16
NPAIR = HL // 2           # head pairs per core = 8


def _build_nc(t_steps: int):
    import concourse.bass as bass
    import concourse.bacc as bacc
    import concourse.mybir as mybir
    import concourse.tile as tile
    from contextlib import ExitStack

    fp32 = mybir.dt.float32
    bf16 = mybir.dt.bfloat16
    nc = bacc.Bacc("TRN2", target_bir_lowering=False, debug=False,
                   num_devices=NCORES)

    xT_d = nc.dram_tensor("xT", [C, t_steps, B], bf16, kind="ExternalInput")
    wi_d = nc.dram_tensor("wi", [C, NPAIR, 128], bf16, kind="ExternalInput")
    wa_d = nc.dram_tensor("wa", [128, NPAIR, 128], bf16, kind="ExternalInput")
    we_d = nc.dram_tensor("we", [128, NPAIR, 128], bf16, kind="ExternalInput")
    on_d = nc.dram_tensor("ones16", [128, NPAIR, 48], bf16, kind="ExternalInput")
    sel_d = nc.dram_tensor("sel", [HL, NPAIR, 128], bf16, kind="ExternalInput")
    out_d = nc.dram_tensor("out", [t_steps, HL, B], fp32, kind="ExternalOutput")

    with ExitStack() as ctx:
        ctx.enter_context(nc.allow_low_precision(reason="bf16 datapath"))
        tc = ctx.enter_context(tile.TileContext(nc))
        singles = ctx.enter_context(tc.tile_pool(name="singles", bufs=1))
        state = ctx.enter_context(tc.tile_pool(name="state", bufs=3))
        xpool = ctx.enter_context(tc.tile_pool(name="xpool", bufs=4))
        vpool = ctx.enter_context(tc.tile_pool(name="vpool", bufs=3))
        upool = ctx.enter_context(tc.tile_pool(name="upool", bufs=3))
        wpool = ctx.enter_context(tc.tile_pool(name="wpool", bufs=3))
        spool = ctx.enter_context(tc.tile_pool(name="spool", bufs=3))
        opool = ctx.enter_context(tc.tile_pool(name="opool", bufs=3))
        ps_pre = ctx.enter_context(tc.tile_pool(name="ps_pre", bufs=2, space="PSUM"))
        ps_e = ctx.enter_context(tc.tile_pool(name="ps_e", bufs=2, space="PSUM"))
        ps_sn = ctx.enter_context(tc.tile_pool(name="ps_sn", bufs=1, space="PSUM"))
        ps_bc = ctx.enter_context(tc.tile_pool(name="ps_bc", bufs=2, space="PSUM"))

        wi_sb = singles.tile([C, NPAIR, 128], bf16)
        wa_sb = singles.tile([128, NPAIR, 128], bf16)
        we_sb = singles.tile([128, NPAIR, 128], bf16)
        on_sb = singles.tile([128, NPAIR, 48], bf16)
        sel_sb = singles.tile([HL, NPAIR, 128], bf16)
        nc.sync.dma_start(out=wi_sb, in_=wi_d[:])
        nc.sync.dma_start(out=wa_sb, in_=wa_d[:])
        nc.sync.dma_start(out=we_sb, in_=we_d[:])
        nc.sync.dma_start(out=on_sb, in_=on_d[:])
        nc.sync.dma_start(out=sel_sb, in_=sel_d[:])

        att = state.tile([128, NPAIR, B], bf16, tag="att")
        nc.vector.memset(att, 1.0 / C)

        for t in range(t_steps):
            xdup = xpool.tile([128, B], bf16)
            nc.sync.dma_start(out=xdup[0:C, :], in_=xT_d[:, t, :])
            nc.sync.dma_start(out=xdup[C:128, :], in_=xT_d[:, t, :])

            v_sb = vpool.tile([128, NPAIR, B], bf16)
            for j in range(NPAIR):
                pre = ps_pre.tile([128, B], fp32)
                nc.tensor.matmul(pre, wi_sb[:, j, :], xdup[0:C, :],
                                 start=True, stop=False)
                nc.tensor.matmul(pre, wa_sb[:, j, :], att[:, j, :],
                                 start=False, stop=True)
                nc.scalar.activation(v_sb[:, j, :], pre,
                                     mybir.ActivationFunctionType.Tanh)

            u_sb = upool.tile([128, NPAIR, B], bf16)
            for j in range(NPAIR):
                e = ps_e.tile([128, B], fp32)
                nc.tensor.matmul(e, we_sb[:, j, :], v_sb[:, j, :],
                                 start=True, stop=True)
                nc.scalar.activation(u_sb[:, j, :], e,
                                     mybir.ActivationFunctionType.Exp)

            # w = u * x  (numerator operand; off the recurrence, on GPSIMD;
            # the last two pairs go to DVE after the reciprocal, where they
            # fill the idle window while bc matmuls run on PE)
            w_sb = wpool.tile([128, NPAIR, B], bf16)
            for j in range(NPAIR - 2):
                nc.gpsimd.tensor_mul(w_sb[:, j, :], u_sb[:, j, :], xdup)

            S_ps = ps_sn.tile([HL, B], fp32, tag="S")
            for j in range(NPAIR):
                nc.tensor.matmul(S_ps, on_sb[:, j, 0:HL], u_sb[:, j, :],
                                 start=(j == 0), stop=(j == NPAIR - 1))

            # approx 1/S emitted directly as bf16 (the fp32 bit-trick is on
            # the input side only; the write port downcasts) - skips a cast.
            from concourse.dve_ops import (RECIP_APPROX_FAST_CONSTS,
                                           RECIPROCAL_APPROX_FAST)
            _rc = RECIP_APPROX_FAST_CONSTS
            rSb = spool.tile([HL, B], bf16, tag="rSb")
            nc.vector._custom_dve(RECIPROCAL_APPROX_FAST, out=rSb, in0=S_ps,
                                  s0=_rc["s0"], s1=_rc["s1"], imm2=_rc["imm2"])

            nc.vector.tensor_mul(w_sb[:, NPAIR - 2, :],
                                 u_sb[:, NPAIR - 2, :], xdup)
            nc.vector.tensor_mul(w_sb[:, NPAIR - 1, :],
                                 u_sb[:, NPAIR - 1, :], xdup)

            num_ps = ps_sn.tile([HL, B], fp32, tag="num")
            for j in range(NPAIR):
                nc.tensor.matmul(num_ps, on_sb[:, j, 32:48], w_sb[:, j, :],
                                 start=(j == 0), stop=(j == NPAIR - 1))

            # att' = u * bcast(rS); bcast via selector matmul per pair
            att_new = state.tile([128, NPAIR, B], bf16, tag="att")
            for j in range(NPAIR):
                bc = ps_bc.tile([128, B], fp32)
                nc.tensor.matmul(bc, sel_sb[:, j, :], rSb,
                                 start=True, stop=True)
                nc.vector.tensor_mul(att_new[:, j, :], u_sb[:, j, :], bc)
            att = att_new

            # out-mul emitted after the att'-muls: it depends on the late
            # num matmuls and must not block the critical DVE chain.
            outb = opool.tile([HL, B], fp32)
            nc.vector.tensor_mul(outb, num_ps, rSb)
            nc.sync.dma_start(out=out_d[t], in_=outb)

    nc.compile()
    return nc


def _host_prep(x, weight_att, weight_input, weight_e):
    """Build per-core input maps (host-side layout prep)."""
    bf = ml_dtypes.bfloat16
    xT = np.ascontiguousarray(x.transpose(2, 1, 0)).astype(bf)  # [C, T, B]

    in_maps = []
    for g in range(NCORES):
        h0 = g * HL
        wi = np.zeros((C, NPAIR, 128), np.float32)
        wa = np.zeros((128, NPAIR, 128), np.float32)
        we = np.zeros((128, NPAIR, 128), np.float32)
        on = np.zeros((128, NPAIR, 48), np.float32)
        sel = np.zeros((HL, NPAIR, 128), np.float32)
        for j in range(NPAIR):
            ha, hb = h0 + 2 * j, h0 + 2 * j + 1
            # lhsT[k, m] = W[h, m, k]
            wi[:, j, 0:C] = weight_input[ha].T
            wi[:, j, C:128] = weight_input[hb].T
            wa[0:C, j, 0:C] = weight_att[ha].T
            wa[C:128, j, C:128] = weight_att[hb].T
            we[0:C, j, 0:C] = weight_e[ha].T
            we[C:128, j, C:128] = weight_e[hb].T
            on[0:C, j, 2 * j] = 1.0
            on[C:128, j, 2 * j + 1] = 1.0
            on[0:C, j, 32 + 2 * j] = 1.0
            on[C:128, j, 32 + 2 * j + 1] = 1.0
            sel[2 * j, j, 0:C] = 1.0
            sel[2 * j + 1, j, C:128] = 1.0
        in_maps.append({
            "xT": xT, "wi": wi.astype(bf), "wa": wa.astype(bf),
            "we": we.astype(bf), "ones16": on.astype(bf),
            "sel": sel.astype(bf),
        })
    return in_maps


def run(x, weight_att, weight_input, weight_e, t_steps=T, trace=False):
    from concourse.bass_utils import run_bass_kernel_spmd

    nc = _build_nc(t_steps)
    in_maps = _host_prep(x, weight_att, weight_input, weight_e)
    if t_steps != T:
        for m in in_maps:
            m["xT"] = np.ascontiguousarray(m["xT"][:, :t_steps, :])
    res = run_bass_kernel_spmd(nc, in_maps, list(range(NCORES)), trace=trace)

    # results[g]["out"]: [t_steps, HL, B] -> out[b, t, g*HL + i]
    out = np.empty((B, t_steps, H), np.float32)
    for g in range(NCORES):
        og = res.results[g]["out"]
        out[:, :, g * HL:(g + 1) * HL] = og.transpose(2, 0, 1)
    return out, res


def kernel(x, weight_att, weight_input, weight_e):
    out, _ = run(x, weight_att, weight_input, weight_e)
    return out


# revision 22
# speedup vs baseline: 1.3269x; 1.2401x over previous
"""Trainium2 Bass kernel for nn_Attention_80805514707533.

Recurrent attention scan: B=512, T=512, C=64, H=128.
Sharding: H across 8 cores (16 heads each); full batch B=512 rides the
matmul moving dimension. C=64 lives on partitions; heads are packed in
pairs (2 x 64 = 128 partitions) with block-diagonal stationary weights.

bf16 datapath (PE 1 cyc/row + fast weight load), fp32 PSUM accumulation,
fast approximate reciprocal for the softmax denominator. Small-K matmuls
(wi: K=64, bc: K=16) are issued in pairs at different PE row groups so
they execute concurrently in the 128x128 array.

Per step t (per core, j = head-pair 0..7):
  pre[j]  = Wi_cat[j].T @ xT_t  +  Wa_blk[j].T @ att[:,j,:]      (PSUM)
  v[j]    = tanh(pre[j])                                          (ACT)
  e[j]    = We_blk[j].T @ v[j]                                    (PSUM)
  u[j]    = exp(e[j])                                             (ACT)
  S       = sum_c u   via ones16 matmuls  -> [16, B]              (PE)
  rS      = 1/S  (approx, fp32)                                   (DVE)
  att'    = u * bcast(rS)   (bcast via selector matmul)           (PE+DVE)
  p       = att' * x                                              (GPSIMD)
  out_t   = sum_c p  via ones16 matmuls -> [16, B] -> DRAM        (PE+DVE)
"""

import numpy as np
import ml_dtypes

B, T, C, H = 512, 512, 64, 128
NCORES = 8
HL = H // NCORES          # heads per core = 16
NPAIR = HL // 2           # head pairs per core = 8


def _build_nc(t_steps: int):
    import concourse.bass as bass
    import concourse.bacc as bacc
    import concourse.mybir as mybir
    import concourse.tile as tile
    from contextlib import ExitStack

    fp32 = mybir.dt.float32
    bf16 = mybir.dt.bfloat16
    nc = bacc.Bacc("TRN2", target_bir_lowering=False, debug=False,
                   num_devices=NCORES)

    xT_d = nc.dram_tensor("xT", [C, t_steps, B], bf16, kind="ExternalInput")
    wi_d = nc.dram_tensor("wi", [C, NPAIR, 128], bf16, kind="ExternalInput")
    wa_d = nc.dram_tensor("wa", [128, NPAIR, 128], bf16, kind="ExternalInput")
    we_d = nc.dram_tensor("we", [128, NPAIR, 128], bf16, kind="ExternalInput")
    on_d = nc.dram_tensor("ones16", [128, NPAIR, 48], bf16, kind="ExternalInput")
    sel_d = nc.dram_tensor("sel", [8, NPAIR, 128], bf16, kind="ExternalInput")
    out_d = nc.dram_tensor("out", [t_steps, HL, B], fp32, kind="ExternalOutput")

    with ExitStack() as ctx:
        ctx.enter_context(nc.allow_low_precision(reason="bf16 datapath"))
        tc = ctx.enter_context(tile.TileContext(nc))
        singles = ctx.enter_context(tc.tile_pool(name="singles", bufs=1))
        state = ctx.enter_context(tc.tile_pool(name="state", bufs=3))
        xpool = ctx.enter_context(tc.tile_pool(name="xpool", bufs=4))
        vpool = ctx.enter_context(tc.tile_pool(name="vpool", bufs=3))
        upool = ctx.enter_context(tc.tile_pool(name="upool", bufs=3))
        wpool = ctx.enter_context(tc.tile_pool(name="wpool", bufs=3))
        spool = ctx.enter_context(tc.tile_pool(name="spool", bufs=3))
        opool = ctx.enter_context(tc.tile_pool(name="opool", bufs=3))
        ps_pre = ctx.enter_context(tc.tile_pool(name="ps_pre", bufs=2, space="PSUM"))
        ps_e = ctx.enter_context(tc.tile_pool(name="ps_e", bufs=2, space="PSUM"))
        ps_sn = ctx.enter_context(tc.tile_pool(name="ps_sn", bufs=1, space="PSUM"))
        ps_bc = ctx.enter_context(tc.tile_pool(name="ps_bc", bufs=2, space="PSUM"))

        wi_sb = singles.tile([C, NPAIR, 128], bf16)
        wa_sb = singles.tile([128, NPAIR, 128], bf16)
        we_sb = singles.tile([128, NPAIR, 128], bf16)
        on_sb = singles.tile([128, NPAIR, 48], bf16)
        sel_sb = singles.tile([8, NPAIR, 128], bf16)
        nc.sync.dma_start(out=wi_sb, in_=wi_d[:])
        nc.sync.dma_start(out=wa_sb, in_=wa_d[:])
        nc.sync.dma_start(out=we_sb, in_=we_d[:])
        nc.sync.dma_start(out=on_sb, in_=on_d[:])
        nc.sync.dma_start(out=sel_sb, in_=sel_d[:])

        att = state.tile([128, NPAIR, B], bf16, tag="att")
        nc.vector.memset(att, 1.0 / C)

        for t in range(t_steps):
            xdup = xpool.tile([128, B], bf16)
            nc.sync.dma_start(out=xdup[0:C, :], in_=xT_d[:, t, :])
            nc.sync.dma_start(out=xdup[C:128, :], in_=xT_d[:, t, :])

            v_sb = vpool.tile([128, NPAIR, B], bf16)
            for j in range(NPAIR):
                pre = ps_pre.tile([128, B], fp32)
                nc.tensor.matmul(pre, wi_sb[:, j, :], xdup[0:C, :],
                                 start=True, stop=False)
                nc.tensor.matmul(pre, wa_sb[:, j, :], att[:, j, :],
                                 start=False, stop=True)
                nc.scalar.activation(v_sb[:, j, :], pre,
                                     mybir.ActivationFunctionType.Tanh)

            u_sb = upool.tile([128, NPAIR, B], bf16)
            for j in range(NPAIR):
                e = ps_e.tile([128, B], fp32)
                nc.tensor.matmul(e, we_sb[:, j, :], v_sb[:, j, :],
                                 start=True, stop=True)
                nc.scalar.activation(u_sb[:, j, :], e,
                                     mybir.ActivationFunctionType.Exp)

            # w = u * x (numerator operand; off the recurrence, on GPSIMD;
            # the last two pairs go to DVE after recip_B)
            w_sb = wpool.tile([128, NPAIR, B], bf16)
            for j in range(NPAIR - 2):
                nc.gpsimd.tensor_mul(w_sb[:, j, :], u_sb[:, j, :], xdup)

            from concourse.dve_ops import (RECIP_APPROX_FAST_CONSTS,
                                           RECIPROCAL_APPROX_FAST)
            _rc = RECIP_APPROX_FAST_CONSTS
            att_new = state.tile([128, NPAIR, B], bf16, tag="att")
            GRP = NPAIR // 2

            # --- group A (pairs 0-3): S, 1/S, bc+att' while B still in exp
            S_A = ps_sn.tile([2 * GRP, B], fp32, tag="sn_a")
            for j in range(GRP):
                nc.tensor.matmul(S_A, on_sb[:, j, 0:8], u_sb[:, j, :],
                                 start=(j == 0), stop=(j == GRP - 1))
            rSb_A = spool.tile([2 * GRP, B], bf16, tag="rSb_A")
            nc.vector._custom_dve(RECIPROCAL_APPROX_FAST, out=rSb_A, in0=S_A,
                                  s0=_rc["s0"], s1=_rc["s1"], imm2=_rc["imm2"])
            for j in range(GRP):
                bc = ps_bc.tile([128, B], fp32)
                nc.tensor.matmul(bc, sel_sb[:, j, :], rSb_A,
                                 start=True, stop=True)
                nc.vector.tensor_mul(att_new[:, j, :], u_sb[:, j, :], bc)

            # --- group B (pairs 4-7)
            S_B = ps_sn.tile([2 * GRP, B], fp32, tag="sn_b")
            for j in range(GRP, NPAIR):
                jj = j - GRP
                nc.tensor.matmul(S_B, on_sb[:, j, 8:16], u_sb[:, j, :],
                                 start=(jj == 0), stop=(jj == GRP - 1))
            rSb_B = spool.tile([2 * GRP, B], bf16, tag="rSb_B")
            nc.vector._custom_dve(RECIPROCAL_APPROX_FAST, out=rSb_B, in0=S_B,
                                  s0=_rc["s0"], s1=_rc["s1"], imm2=_rc["imm2"])
            nc.vector.tensor_mul(w_sb[:, NPAIR - 2, :],
                                 u_sb[:, NPAIR - 2, :], xdup)
            nc.vector.tensor_mul(w_sb[:, NPAIR - 1, :],
                                 u_sb[:, NPAIR - 1, :], xdup)
            for j in range(GRP, NPAIR):
                bc = ps_bc.tile([128, B], fp32)
                nc.tensor.matmul(bc, sel_sb[:, j, :], rSb_B,
                                 start=True, stop=True)
                nc.vector.tensor_mul(att_new[:, j, :], u_sb[:, j, :], bc)
            att = att_new

            # --- output: per-group num reduction and rescale (banks shared
            # with S via tag rotation; the recip reads complete early)
            num_A = ps_sn.tile([2 * GRP, B], fp32, tag="sn_a")
            for j in range(GRP):
                nc.tensor.matmul(num_A, on_sb[:, j, 32:40], w_sb[:, j, :],
                                 start=(j == 0), stop=(j == GRP - 1))
            num_B = ps_sn.tile([2 * GRP, B], fp32, tag="sn_b")
            for j in range(GRP, NPAIR):
                jj = j - GRP
                nc.tensor.matmul(num_B, on_sb[:, j, 40:48], w_sb[:, j, :],
                                 start=(jj == 0), stop=(jj == GRP - 1))
            outb_A = opool.tile([2 * GRP, B], fp32, tag="outb_A")
            nc.vector.tensor_mul(outb_A, num_A, rSb_A)
            nc.sync.dma_start(out=out_d[t, 0:2 * GRP, :], in_=outb_A)
            outb_B = opool.tile([2 * GRP, B], fp32, tag="outb_B")
            nc.vector.tensor_mul(outb_B, num_B, rSb_B)
            nc.sync.dma_start(out=out_d[t, 2 * GRP:HL, :], in_=outb_B)

    nc.compile()
    return nc


def _host_prep(x, weight_att, weight_input, weight_e):
    """Build per-core input maps (host-side layout prep)."""
    bf = ml_dtypes.bfloat16
    xT = np.ascontiguousarray(x.transpose(2, 1, 0)).astype(bf)  # [C, T, B]

    in_maps = []
    for g in range(NCORES):
        h0 = g * HL
        wi = np.zeros((C, NPAIR, 128), np.float32)
        wa = np.zeros((128, NPAIR, 128), np.float32)
        we = np.zeros((128, NPAIR, 128), np.float32)
        on = np.zeros((128, NPAIR, 48), np.float32)
        sel = np.zeros((8, NPAIR, 128), np.float32)
        for j in range(NPAIR):
            ha, hb = h0 + 2 * j, h0 + 2 * j + 1
            # lhsT[k, m] = W[h, m, k]
            wi[:, j, 0:C] = weight_input[ha].T
            wi[:, j, C:128] = weight_input[hb].T
            wa[0:C, j, 0:C] = weight_att[ha].T
            wa[C:128, j, C:128] = weight_att[hb].T
            we[0:C, j, 0:C] = weight_e[ha].T
            we[C:128, j, C:128] = weight_e[hb].T
            on[0:C, j, 2 * j] = 1.0
            on[C:128, j, 2 * j + 1] = 1.0
            on[0:C, j, 32 + 2 * j] = 1.0
            on[C:128, j, 32 + 2 * j + 1] = 1.0
            sel[2 * (j % 4), j, 0:C] = 1.0
            sel[2 * (j % 4) + 1, j, C:128] = 1.0
        in_maps.append({
            "xT": xT, "wi": wi.astype(bf), "wa": wa.astype(bf),
            "we": we.astype(bf), "ones16": on.astype(bf),
            "sel": sel.astype(bf),
        })
    return in_maps


def run(x, weight_att, weight_input, weight_e, t_steps=T, trace=False):
    from concourse.bass_utils import run_bass_kernel_spmd

    nc = _build_nc(t_steps)
    in_maps = _host_prep(x, weight_att, weight_input, weight_e)
    if t_steps != T:
        for m in in_maps:
            m["xT"] = np.ascontiguousarray(m["xT"][:, :t_steps, :])
    res = run_bass_kernel_spmd(nc, in_maps, list(range(NCORES)), trace=trace)

    # results[g]["out"]: [t_steps, HL, B] -> out[b, t, g*HL + i]
    out = np.empty((B, t_steps, H), np.float32)
    for g in range(NCORES):
        og = res.results[g]["out"]
        out[:, :, g * HL:(g + 1) * HL] = og.transpose(2, 0, 1)
    return out, res


def kernel(x, weight_att, weight_input, weight_e):
    out, _ = run(x, weight_att, weight_input, weight_e)
    return out


# revision 24
# speedup vs baseline: 1.5920x; 1.1998x over previous
"""Trainium2 Bass kernel for nn_Attention_80805514707533.

Recurrent attention scan: B=512, T=512, C=64, H=128.
Sharding: H across 8 cores (16 heads each); full batch B=512 rides the
matmul moving dimension. C=64 lives on partitions; heads are packed in
pairs (2 x 64 = 128 partitions) with block-diagonal stationary weights.

bf16 datapath (PE 1 cyc/row + fast weight load), fp32 PSUM accumulation.
The softmax normalization is pipelined in two 4-pair groups: pairs 0-3
compute S, the approximate 1/S (a custom DVE op emitting bf16 directly),
and their bc/att' updates while pairs 4-7 are still in their exp phase.
This keeps the tensor engine dense enough that the HAM clock gate holds
2.4GHz for the whole run instead of oscillating down to 1.2GHz.

Per step t (per core, j = head-pair 0..7, groups A=0-3 / B=4-7):
  pre[j]  = Wi_cat[j].T @ xT_t  +  Wa_blk[j].T @ att[:,j,:]      (PSUM)
  v[j]    = tanh(pre[j])                                          (ACT)
  e[j]    = We_blk[j].T @ v[j]                                    (PSUM)
  u[j]    = exp(e[j])                                             (ACT)
  w[j]    = u[j] * x    (pairs 0-5 GPSIMD; 6-7 DVE after recip_B)
  S_g     = sum_c u  per group via ones matmuls -> [8, B]         (PE)
  rS_g    = 1/S_g  (approx, bf16 out)                             (DVE)
  att'[j] = u[j] * bcast(rS_g)  (bcast via selector matmul)       (PE+DVE)
  num_g   = sum_c w per group -> [8, B]                           (PE)
  out_t   = num_g * rS_g -> DRAM                                  (DVE)
"""

import numpy as np
import ml_dtypes

B, T, C, H = 512, 512, 64, 128
NCORES = 8
HL = H // NCORES          # heads per core = 16
NPAIR = HL // 2           # head pairs per core = 8


def _build_nc(t_steps: int):
    import concourse.bass as bass
    import concourse.bacc as bacc
    import concourse.mybir as mybir
    import concourse.tile as tile
    from contextlib import ExitStack

    fp32 = mybir.dt.float32
    bf16 = mybir.dt.bfloat16
    nc = bacc.Bacc("TRN2", target_bir_lowering=False, debug=False,
                   num_devices=NCORES)

    xT_d = nc.dram_tensor("xT", [C, t_steps, B], bf16, kind="ExternalInput")
    wi_d = nc.dram_tensor("wi", [128, NPAIR // 2, 128], bf16, kind="ExternalInput")
    wa_d = nc.dram_tensor("wa", [128, NPAIR, 128], bf16, kind="ExternalInput")
    we_d = nc.dram_tensor("we", [128, NPAIR, 128], bf16, kind="ExternalInput")
    on_d = nc.dram_tensor("ones16", [128, NPAIR, 48], bf16, kind="ExternalInput")
    sel_d = nc.dram_tensor("sel", [8, NPAIR, 128], bf16, kind="ExternalInput")
    out_d = nc.dram_tensor("out", [t_steps, HL, B], fp32, kind="ExternalOutput")

    with ExitStack() as ctx:
        ctx.enter_context(nc.allow_low_precision(reason="bf16 datapath"))
        tc = ctx.enter_context(tile.TileContext(nc))
        singles = ctx.enter_context(tc.tile_pool(name="singles", bufs=1))
        state = ctx.enter_context(tc.tile_pool(name="state", bufs=3))
        xpool = ctx.enter_context(tc.tile_pool(name="xpool", bufs=4))
        vpool = ctx.enter_context(tc.tile_pool(name="vpool", bufs=3))
        upool = ctx.enter_context(tc.tile_pool(name="upool", bufs=3))
        wpool = ctx.enter_context(tc.tile_pool(name="wpool", bufs=3))
        spool = ctx.enter_context(tc.tile_pool(name="spool", bufs=3))
        opool = ctx.enter_context(tc.tile_pool(name="opool", bufs=3))
        ps_pre = ctx.enter_context(tc.tile_pool(name="ps_pre", bufs=1, space="PSUM"))
        ps_e = ctx.enter_context(tc.tile_pool(name="ps_e", bufs=2, space="PSUM"))
        ps_sn = ctx.enter_context(tc.tile_pool(name="ps_sn", bufs=1, space="PSUM"))
        ps_bc = ctx.enter_context(tc.tile_pool(name="ps_bc", bufs=2, space="PSUM"))

        wi_sb = singles.tile([128, NPAIR // 2, 128], bf16)
        wa_sb = singles.tile([128, NPAIR, 128], bf16)
        we_sb = singles.tile([128, NPAIR, 128], bf16)
        on_sb = singles.tile([128, NPAIR, 48], bf16)
        sel_sb = singles.tile([8, NPAIR, 128], bf16)
        nc.sync.dma_start(out=wi_sb, in_=wi_d[:])
        nc.sync.dma_start(out=wa_sb, in_=wa_d[:])
        nc.sync.dma_start(out=we_sb, in_=we_d[:])
        nc.sync.dma_start(out=on_sb, in_=on_d[:])
        nc.sync.dma_start(out=sel_sb, in_=sel_d[:])

        att = state.tile([128, NPAIR, B], bf16, tag="att")
        nc.vector.memset(att, 1.0 / C)

        for t in range(t_steps):
            xdup = xpool.tile([128, B], bf16)
            nc.sync.dma_start(out=xdup[0:C, :], in_=xT_d[:, t, :])
            nc.sync.dma_start(out=xdup[C:128, :], in_=xT_d[:, t, :])

            v_sb = vpool.tile([128, NPAIR, B], bf16)
            for i in range(NPAIR // 2):
                ja, jb = 2 * i, 2 * i + 1
                pre_a = ps_pre.tile([128, B], fp32, tag="pre_a")
                pre_b = ps_pre.tile([128, B], fp32, tag="pre_b")
                # K=64 wi matmuls issued back-to-back at PE row groups
                # 0-63 / 64-127 (x is replicated in both halves of xdup)
                nc.tensor.matmul(pre_a, wi_sb[0:64, i, :], xdup[0:64, :],
                                 start=True, stop=False)
                nc.tensor.matmul(pre_b, wi_sb[64:128, i, :], xdup[64:128, :],
                                 start=True, stop=False)
                nc.tensor.matmul(pre_a, wa_sb[:, ja, :], att[:, ja, :],
                                 start=False, stop=True)
                nc.scalar.activation(v_sb[:, ja, :], pre_a,
                                     mybir.ActivationFunctionType.Tanh)
                nc.tensor.matmul(pre_b, wa_sb[:, jb, :], att[:, jb, :],
                                 start=False, stop=True)
                nc.scalar.activation(v_sb[:, jb, :], pre_b,
                                     mybir.ActivationFunctionType.Tanh)

            u_sb = upool.tile([128, NPAIR, B], bf16)
            for j in range(NPAIR):
                e = ps_e.tile([128, B], fp32)
                nc.tensor.matmul(e, we_sb[:, j, :], v_sb[:, j, :],
                                 start=True, stop=True)
                nc.scalar.activation(u_sb[:, j, :], e,
                                     mybir.ActivationFunctionType.Exp)

            # w = u * x (numerator operand; off the recurrence, on GPSIMD;
            # the last two pairs go to DVE after recip_B)
            w_sb = wpool.tile([128, NPAIR, B], bf16)
            for j in range(NPAIR - 2):
                nc.gpsimd.tensor_mul(w_sb[:, j, :], u_sb[:, j, :], xdup)

            from concourse.dve_ops import (RECIP_APPROX_FAST_CONSTS,
                                           RECIPROCAL_APPROX_FAST)
            _rc = RECIP_APPROX_FAST_CONSTS
            att_new = state.tile([128, NPAIR, B], bf16, tag="att")
            GRP = NPAIR // 2

            # --- group A (pairs 0-3): S, 1/S, bc+att' while B still in exp
            S_A = ps_sn.tile([2 * GRP, B], fp32, tag="sn_a")
            for j in range(GRP):
                nc.tensor.matmul(S_A, on_sb[:, j, 0:8], u_sb[:, j, :],
                                 start=(j == 0), stop=(j == GRP - 1))
            rSb_A = spool.tile([2 * GRP, B], bf16, tag="rSb_A")
            nc.vector._custom_dve(RECIPROCAL_APPROX_FAST, out=rSb_A, in0=S_A,
                                  s0=_rc["s0"], s1=_rc["s1"], imm2=_rc["imm2"])
            for j in range(GRP):
                bc = ps_bc.tile([128, B], fp32)
                nc.tensor.matmul(bc, sel_sb[:, j, :], rSb_A,
                                 start=True, stop=True)
                nc.vector.tensor_mul(att_new[:, j, :], u_sb[:, j, :], bc)

            # --- group B (pairs 4-7)
            S_B = ps_sn.tile([2 * GRP, B], fp32, tag="sn_b")
            for j in range(GRP, NPAIR):
                jj = j - GRP
                nc.tensor.matmul(S_B, on_sb[:, j, 8:16], u_sb[:, j, :],
                                 start=(jj == 0), stop=(jj == GRP - 1))
            rSb_B = spool.tile([2 * GRP, B], bf16, tag="rSb_B")
            nc.vector._custom_dve(RECIPROCAL_APPROX_FAST, out=rSb_B, in0=S_B,
                                  s0=_rc["s0"], s1=_rc["s1"], imm2=_rc["imm2"])
            nc.vector.tensor_mul(w_sb[:, NPAIR - 2, :],
                                 u_sb[:, NPAIR - 2, :], xdup)
            nc.vector.tensor_mul(w_sb[:, NPAIR - 1, :],
                                 u_sb[:, NPAIR - 1, :], xdup)
            for j in range(GRP, NPAIR):
                bc = ps_bc.tile([128, B], fp32)
                nc.tensor.matmul(bc, sel_sb[:, j, :], rSb_B,
                                 start=True, stop=True)
                nc.vector.tensor_mul(att_new[:, j, :], u_sb[:, j, :], bc)
            att = att_new

            # --- output: per-group num reduction and rescale (banks shared
            # with S via tag rotation; the recip reads complete early)
            num_A = ps_sn.tile([2 * GRP, B], fp32, tag="sn_a")
            for j in range(GRP):
                nc.tensor.matmul(num_A, on_sb[:, j, 32:40], w_sb[:, j, :],
                                 start=(j == 0), stop=(j == GRP - 1))
            num_B = ps_sn.tile([2 * GRP, B], fp32, tag="sn_b")
            for j in range(GRP, NPAIR):
                jj = j - GRP
                nc.tensor.matmul(num_B, on_sb[:, j, 40:48], w_sb[:, j, :],
                                 start=(jj == 0), stop=(jj == GRP - 1))
            outb_A = opool.tile([2 * GRP, B], fp32, tag="outb_A")
            nc.vector.tensor_mul(outb_A, num_A, rSb_A)
            nc.sync.dma_start(out=out_d[t, 0:2 * GRP, :], in_=outb_A)
            outb_B = opool.tile([2 * GRP, B], fp32, tag="outb_B")
            nc.vector.tensor_mul(outb_B, num_B, rSb_B)
            nc.sync.dma_start(out=out_d[t, 2 * GRP:HL, :], in_=outb_B)

    nc.compile()
    return nc


def _host_prep(x, weight_att, weight_input, weight_e):
    """Build per-core input maps (host-side layout prep)."""
    bf = ml_dtypes.bfloat16
    xT = np.ascontiguousarray(x.transpose(2, 1, 0)).astype(bf)  # [C, T, B]

    in_maps = []
    for g in range(NCORES):
        h0 = g * HL
        wi = np.zeros((128, NPAIR // 2, 128), np.float32)
        wa = np.zeros((128, NPAIR, 128), np.float32)
        we = np.zeros((128, NPAIR, 128), np.float32)
        on = np.zeros((128, NPAIR, 48), np.float32)
        sel = np.zeros((8, NPAIR, 128), np.float32)
        for j in range(NPAIR):
            ha, hb = h0 + 2 * j, h0 + 2 * j + 1
            # lhsT[k, m] = W[h, m, k]; wi pairs stacked on PE row halves
            half = 64 * (j % 2)
            wi[half:half + C, j // 2, 0:C] = weight_input[ha].T
            wi[half:half + C, j // 2, C:128] = weight_input[hb].T
            wa[0:C, j, 0:C] = weight_att[ha].T
            wa[C:128, j, C:128] = weight_att[hb].T
            we[0:C, j, 0:C] = weight_e[ha].T
            we[C:128, j, C:128] = weight_e[hb].T
            on[0:C, j, 2 * j] = 1.0
            on[C:128, j, 2 * j + 1] = 1.0
            on[0:C, j, 32 + 2 * j] = 1.0
            on[C:128, j, 32 + 2 * j + 1] = 1.0
            sel[2 * (j % 4), j, 0:C] = 1.0
            sel[2 * (j % 4) + 1, j, C:128] = 1.0
        in_maps.append({
            "xT": xT, "wi": wi.astype(bf), "wa": wa.astype(bf),
            "we": we.astype(bf), "ones16": on.astype(bf),
            "sel": sel.astype(bf),
        })
    return in_maps


def run(x, weight_att, weight_input, weight_e, t_steps=T, trace=False):
    from concourse.bass_utils import run_bass_kernel_spmd

    nc = _build_nc(t_steps)
    in_maps = _host_prep(x, weight_att, weight_input, weight_e)
    if t_steps != T:
        for m in in_maps:
            m["xT"] = np.ascontiguousarray(m["xT"][:, :t_steps, :])
    res = run_bass_kernel_spmd(nc, in_maps, list(range(NCORES)), trace=trace)

    # results[g]["out"]: [t_steps, HL, B] -> out[b, t, g*HL + i]
    out = np.empty((B, t_steps, H), np.float32)
    for g in range(NCORES):
        og = res.results[g]["out"]
        out[:, :, g * HL:(g + 1) * HL] = og.transpose(2, 0, 1)
    return out, res


def kernel(x, weight_att, weight_input, weight_e):
    out, _ = run(x, weight_att, weight_input, weight_e)
    return out


# revision 25
# speedup vs baseline: 1.6735x; 1.0512x over previous
"""Trainium2 Bass kernel for nn_Attention_80805514707533.

Recurrent attention scan: B=512, T=512, C=64, H=128.
Sharding: H across 8 cores (16 heads each); full batch B=512 rides the
matmul moving dimension. C=64 lives on partitions; heads are packed in
pairs (2 x 64 = 128 partitions) with block-diagonal stationary weights.

bf16 datapath (PE 1 cyc/row + fast weight load), fp32 PSUM accumulation.
The softmax normalization is pipelined in two 4-pair groups: pairs 0-3
compute S, the approximate 1/S (a custom DVE op emitting bf16 directly),
and their bc/att' updates while pairs 4-7 are still in their exp phase.
This keeps the tensor engine dense enough that the HAM clock gate holds
2.4GHz for the whole run instead of oscillating down to 1.2GHz.

Per step t (per core, j = head-pair 0..7, groups A=0-3 / B=4-7):
  pre[j]  = Wi_cat[j].T @ xT_t  +  Wa_blk[j].T @ att[:,j,:]      (PSUM)
  v[j]    = tanh(pre[j])                                          (ACT)
  e[j]    = We_blk[j].T @ v[j]                                    (PSUM)
  u[j]    = exp(e[j])                                             (ACT)
  w[j]    = u[j] * x    (pairs 0-5 GPSIMD; 6-7 DVE after recip_B)
  S_g     = sum_c u  per group via ones matmuls -> [8, B]         (PE)
  rS_g    = 1/S_g  (approx, bf16 out)                             (DVE)
  att'[j] = u[j] * bcast(rS_g)  (bcast via selector matmul)       (PE+DVE)
  num_g   = sum_c w per group -> [8, B]                           (PE)
  out_t   = num_g * rS_g -> DRAM                                  (DVE)
"""

import numpy as np
import ml_dtypes

B, T, C, H = 512, 512, 64, 128
NCORES = 8
HL = H // NCORES          # heads per core = 16
NPAIR = HL // 2           # head pairs per core = 8


def _build_nc(t_steps: int):
    import concourse.bass as bass
    import concourse.bacc as bacc
    import concourse.mybir as mybir
    import concourse.tile as tile
    from contextlib import ExitStack

    fp32 = mybir.dt.float32
    bf16 = mybir.dt.bfloat16
    nc = bacc.Bacc("TRN2", target_bir_lowering=False, debug=False,
                   num_devices=NCORES)

    xT_d = nc.dram_tensor("xT", [C, t_steps, B], bf16, kind="ExternalInput")
    wi_d = nc.dram_tensor("wi", [128, NPAIR // 2, 128], bf16, kind="ExternalInput")
    wa_d = nc.dram_tensor("wa", [128, NPAIR, 128], bf16, kind="ExternalInput")
    we_d = nc.dram_tensor("we", [128, NPAIR, 128], bf16, kind="ExternalInput")
    on_d = nc.dram_tensor("ones16", [128, NPAIR, 48], bf16, kind="ExternalInput")
    sel_d = nc.dram_tensor("sel", [8, NPAIR, 128], bf16, kind="ExternalInput")
    out_d = nc.dram_tensor("out", [t_steps, HL, B], fp32, kind="ExternalOutput")

    with ExitStack() as ctx:
        ctx.enter_context(nc.allow_low_precision(reason="bf16 datapath"))
        tc = ctx.enter_context(tile.TileContext(nc))
        singles = ctx.enter_context(tc.tile_pool(name="singles", bufs=1))
        state = ctx.enter_context(tc.tile_pool(name="state", bufs=3))
        xpool = ctx.enter_context(tc.tile_pool(name="xpool", bufs=4))
        vpool = ctx.enter_context(tc.tile_pool(name="vpool", bufs=3))
        upool = ctx.enter_context(tc.tile_pool(name="upool", bufs=3))
        wpool = ctx.enter_context(tc.tile_pool(name="wpool", bufs=3))
        spool = ctx.enter_context(tc.tile_pool(name="spool", bufs=3))
        opool = ctx.enter_context(tc.tile_pool(name="opool", bufs=3))
        ps_pre = ctx.enter_context(tc.tile_pool(name="ps_pre", bufs=1, space="PSUM"))
        ps_e = ctx.enter_context(tc.tile_pool(name="ps_e", bufs=1, space="PSUM"))
        ps_sn = ctx.enter_context(tc.tile_pool(name="ps_sn", bufs=1, space="PSUM"))
        ps_bc = ctx.enter_context(tc.tile_pool(name="ps_bc", bufs=2, space="PSUM"))

        wi_sb = singles.tile([128, NPAIR // 2, 128], bf16)
        wa_sb = singles.tile([128, NPAIR, 128], bf16)
        we_sb = singles.tile([128, NPAIR, 128], bf16)
        on_sb = singles.tile([128, NPAIR, 48], bf16)
        sel_sb = singles.tile([8, NPAIR, 128], bf16)
        nc.sync.dma_start(out=wi_sb, in_=wi_d[:])
        nc.sync.dma_start(out=wa_sb, in_=wa_d[:])
        nc.sync.dma_start(out=we_sb, in_=we_d[:])
        nc.sync.dma_start(out=on_sb, in_=on_d[:])
        nc.sync.dma_start(out=sel_sb, in_=sel_d[:])

        att = state.tile([128, NPAIR, B], bf16, tag="att")
        nc.vector.memset(att, 1.0 / C)

        for t in range(t_steps):
            xdup = xpool.tile([128, B], bf16)
            nc.sync.dma_start(out=xdup[0:C, :], in_=xT_d[:, t, :])
            nc.sync.dma_start(out=xdup[C:128, :], in_=xT_d[:, t, :])

            v_sb = vpool.tile([128, NPAIR, B], bf16)
            for i in range(NPAIR // 2):
                ja, jb = 2 * i, 2 * i + 1
                pre2 = ps_pre.tile([128, 2, B], fp32, tag="pre")
                # K=64 wi matmuls at PE row halves 0-63 / 64-127 (x is
                # replicated in both halves of xdup); one 2-bank PSUM tile
                # per pair-pair so tanh runs at FD=1024.
                nc.tensor.matmul(pre2[:, 0, :], wi_sb[0:64, i, :],
                                 xdup[0:64, :], start=True, stop=False)
                nc.tensor.matmul(pre2[:, 1, :], wi_sb[64:128, i, :],
                                 xdup[64:128, :], start=True, stop=False)
                nc.tensor.matmul(pre2[:, 0, :], wa_sb[:, ja, :],
                                 att[:, ja, :], start=False, stop=True)
                nc.tensor.matmul(pre2[:, 1, :], wa_sb[:, jb, :],
                                 att[:, jb, :], start=False, stop=True)
                nc.scalar.activation(v_sb[:, ja:jb + 1, :], pre2,
                                     mybir.ActivationFunctionType.Tanh)

            u_sb = upool.tile([128, NPAIR, B], bf16)
            for i in range(NPAIR // 2):
                ja, jb = 2 * i, 2 * i + 1
                e2 = ps_e.tile([128, 2, B], fp32, tag="e")
                nc.tensor.matmul(e2[:, 0, :], we_sb[:, ja, :], v_sb[:, ja, :],
                                 start=True, stop=True)
                nc.tensor.matmul(e2[:, 1, :], we_sb[:, jb, :], v_sb[:, jb, :],
                                 start=True, stop=True)
                nc.scalar.activation(u_sb[:, ja:jb + 1, :], e2,
                                     mybir.ActivationFunctionType.Exp)

            # w = u * x (numerator operand; off the recurrence, on GPSIMD;
            # the last two pairs go to DVE after recip_B)
            w_sb = wpool.tile([128, NPAIR, B], bf16)
            for j in range(NPAIR - 2):
                nc.gpsimd.tensor_mul(w_sb[:, j, :], u_sb[:, j, :], xdup)

            from concourse.dve_ops import (RECIP_APPROX_FAST_CONSTS,
                                           RECIPROCAL_APPROX_FAST)
            _rc = RECIP_APPROX_FAST_CONSTS
            att_new = state.tile([128, NPAIR, B], bf16, tag="att")
            GRP = NPAIR // 2

            # --- group A (pairs 0-3): S, 1/S, bc+att' while B still in exp
            S_A = ps_sn.tile([2 * GRP, B], fp32, tag="sn_a")
            for j in range(GRP):
                nc.tensor.matmul(S_A, on_sb[:, j, 0:8], u_sb[:, j, :],
                                 start=(j == 0), stop=(j == GRP - 1))
            rSb_A = spool.tile([2 * GRP, B], bf16, tag="rSb_A")
            nc.vector._custom_dve(RECIPROCAL_APPROX_FAST, out=rSb_A, in0=S_A,
                                  s0=_rc["s0"], s1=_rc["s1"], imm2=_rc["imm2"])
            for j in range(GRP):
                bc = ps_bc.tile([128, B], fp32)
                nc.tensor.matmul(bc, sel_sb[:, j, :], rSb_A,
                                 start=True, stop=True)
                nc.vector.tensor_mul(att_new[:, j, :], u_sb[:, j, :], bc)

            # --- group B (pairs 4-7)
            S_B = ps_sn.tile([2 * GRP, B], fp32, tag="sn_b")
            for j in range(GRP, NPAIR):
                jj = j - GRP
                nc.tensor.matmul(S_B, on_sb[:, j, 8:16], u_sb[:, j, :],
                                 start=(jj == 0), stop=(jj == GRP - 1))
            rSb_B = spool.tile([2 * GRP, B], bf16, tag="rSb_B")
            nc.vector._custom_dve(RECIPROCAL_APPROX_FAST, out=rSb_B, in0=S_B,
                                  s0=_rc["s0"], s1=_rc["s1"], imm2=_rc["imm2"])
            nc.vector.tensor_mul(w_sb[:, NPAIR - 2, :],
                                 u_sb[:, NPAIR - 2, :], xdup)
            nc.vector.tensor_mul(w_sb[:, NPAIR - 1, :],
                                 u_sb[:, NPAIR - 1, :], xdup)
            for j in range(GRP, NPAIR):
                bc = ps_bc.tile([128, B], fp32)
                nc.tensor.matmul(bc, sel_sb[:, j, :], rSb_B,
                                 start=True, stop=True)
                nc.vector.tensor_mul(att_new[:, j, :], u_sb[:, j, :], bc)
            att = att_new

            # --- output: per-group num reduction and rescale (banks shared
            # with S via tag rotation; the recip reads complete early)
            num_A = ps_sn.tile([2 * GRP, B], fp32, tag="sn_a")
            for j in range(GRP):
                nc.tensor.matmul(num_A, on_sb[:, j, 32:40], w_sb[:, j, :],
                                 start=(j == 0), stop=(j == GRP - 1))
            num_B = ps_sn.tile([2 * GRP, B], fp32, tag="sn_b")
            for j in range(GRP, NPAIR):
                jj = j - GRP
                nc.tensor.matmul(num_B, on_sb[:, j, 40:48], w_sb[:, j, :],
                                 start=(jj == 0), stop=(jj == GRP - 1))
            outb_A = opool.tile([2 * GRP, B], fp32, tag="outb_A")
            nc.vector.tensor_mul(outb_A, num_A, rSb_A)
            nc.sync.dma_start(out=out_d[t, 0:2 * GRP, :], in_=outb_A)
            outb_B = opool.tile([2 * GRP, B], fp32, tag="outb_B")
            nc.vector.tensor_mul(outb_B, num_B, rSb_B)
            nc.sync.dma_start(out=out_d[t, 2 * GRP:HL, :], in_=outb_B)

    nc.compile()
    return nc


def _host_prep(x, weight_att, weight_input, weight_e):
    """Build per-core input maps (host-side layout prep)."""
    bf = ml_dtypes.bfloat16
    xT = np.ascontiguousarray(x.transpose(2, 1, 0)).astype(bf)  # [C, T, B]

    in_maps = []
    for g in range(NCORES):
        h0 = g * HL
        wi = np.zeros((128, NPAIR // 2, 128), np.float32)
        wa = np.zeros((128, NPAIR, 128), np.float32)
        we = np.zeros((128, NPAIR, 128), np.float32)
        on = np.zeros((128, NPAIR, 48), np.float32)
        sel = np.zeros((8, NPAIR, 128), np.float32)
        for j in range(NPAIR):
            ha, hb = h0 + 2 * j, h0 + 2 * j + 1
            # lhsT[k, m] = W[h, m, k]; wi pairs stacked on PE row halves
            half = 64 * (j % 2)
            wi[half:half + C, j // 2, 0:C] = weight_input[ha].T
            wi[half:half + C, j // 2, C:128] = weight_input[hb].T
            wa[0:C, j, 0:C] = weight_att[ha].T
            wa[C:128, j, C:128] = weight_att[hb].T
            we[0:C, j, 0:C] = weight_e[ha].T
            we[C:128, j, C:128] = weight_e[hb].T
            on[0:C, j, 2 * j] = 1.0
            on[C:128, j, 2 * j + 1] = 1.0
            on[0:C, j, 32 + 2 * j] = 1.0
            on[C:128, j, 32 + 2 * j + 1] = 1.0
            sel[2 * (j % 4), j, 0:C] = 1.0
            sel[2 * (j % 4) + 1, j, C:128] = 1.0
        in_maps.append({
            "xT": xT, "wi": wi.astype(bf), "wa": wa.astype(bf),
            "we": we.astype(bf), "ones16": on.astype(bf),
            "sel": sel.astype(bf),
        })
    return in_maps


def run(x, weight_att, weight_input, weight_e, t_steps=T, trace=False):
    from concourse.bass_utils import run_bass_kernel_spmd

    nc = _build_nc(t_steps)
    in_maps = _host_prep(x, weight_att, weight_input, weight_e)
    if t_steps != T:
        for m in in_maps:
            m["xT"] = np.ascontiguousarray(m["xT"][:, :t_steps, :])
    res = run_bass_kernel_spmd(nc, in_maps, list(range(NCORES)), trace=trace)

    # results[g]["out"]: [t_steps, HL, B] -> out[b, t, g*HL + i]
    out = np.empty((B, t_steps, H), np.float32)
    for g in range(NCORES):
        og = res.results[g]["out"]
        out[:, :, g * HL:(g + 1) * HL] = og.transpose(2, 0, 1)
    return out, res


def kernel(x, weight_att, weight_input, weight_e):
    out, _ = run(x, weight_att, weight_input, weight_e)
    return out
